# revision 2
# baseline (speedup 1.0000x reference)
"""TRN2 Bass kernel v2 for BasicLSTM (B=32, T=512, IN=512, H=1024).

Strategy vs v1: the wall time of the recurrence is 512 serial steps x
per-step latency L; chains/batch-splits cannot reduce it, only L can.
So v2 minimizes L with a TRANSPOSED single-chain layout:

  - Core k owns gate columns [i_k | f_k | g_k | o_k] (H-slice k*128..),
    and computes z^T = U_k^T h + xz^T directly: out tiles are
    [128 gate-rows, 32 batch] -- full 128-partition utilization for the
    activations/elementwise, and h^T [128, 32] is produced in exactly
    the layout the broadcast needs (no PE transpose on the serial path).
  - U is the STATIONARY operand (bf16 -> fast weight load), h^T bf16 is
    the moving operand: 4 gate-chunks x 8 K-chunks = 32 small matmuls
    per step, LDW-bound at ~55-70ns each.
  - xz^T is precomputed by phase A into SBUF-resident bf16 (128KB/part):
    no per-step DRAM traffic; injected into each gate's PSUM bank with
    an identity matmul (start=True) emitted a step EARLY so it runs in
    the exchange-wait gap.  Bias is applied for free via the ACT bias
    port (per-partition in the transposed layout).
  - Exchange: remote_dma_broadcast of h^T bf16 [128, 32] (descgen-early
    prep + trigger, cumulative credit counting, wait in tile_critical)
    -- same machinery as v1 but once per step instead of twice.
  - Output: h kept fp32 in a 32-step SBUF ring, bulk-DMA'd to DRAM in
    h^T-major layout [128, T, B]; the host transposes when assembling.
  - Numerics: x/W/xz/U/h in bf16, PSUM/activations/c/h-out in fp32.
    Host-emulated rel err ~5e-3 (gate is 2e-2).
"""

import numpy as np
import ml_dtypes

import concourse.bass as bass
import concourse.mybir as mybir
import concourse.tile as tile
from concourse import bacc, bass_utils
from concourse.bass import ts, ds
from concourse.masks import make_identity
from bass_rust import add_dep_helper

B = 32                # batch (single chain, all rows)
T = 512
IN = 512
H = 1024
NCORES = 8
NS = 4 * H // NCORES  # 512 gate cols per core
HS = H // NCORES      # 128 h cols per core
NG = 4                # gates per core (i, f, g, o), each an HS-chunk
TB = T * B
F32 = mybir.dt.float32
BF16 = mybir.dt.bfloat16
AF = mybir.ActivationFunctionType
RSTEPS = 32           # h output ring: steps buffered before one bulk DMA
PADC = NCORES * B     # pad column in hT_recv (Tile dep threading)


def _build(t_steps: int = T, ablate: frozenset = frozenset(), reps: int = 1,
           sim_local: bool = False):
    """ablate (perf experiments only, breaks numerics):
    'aonly' - phase A only;  'noex' - skip broadcasts + waits (stale hT);
    'nomm'  - skip the recurrent matmuls.
    sim_local: replace the cross-core exchange with a local SBUF DMA so a
    single-core TimelineSim run has an equivalent serial dependency."""
    assert t_steps % 4 == 0
    nc = bacc.Bacc("TRN2", debug=False, num_devices=NCORES)

    x_d = nc.dram_tensor("x", [t_steps * B, IN], BF16, kind="ExternalInput")
    w_d = nc.dram_tensor("w", [IN, NS], BF16, kind="ExternalInput")
    u_d = nc.dram_tensor("u", [H, NS], BF16, kind="ExternalInput")
    b_d = nc.dram_tensor("b", [128, NG], F32, kind="ExternalInput")
    hs_d = nc.dram_tensor("hs", [128, t_steps * B], F32, kind="ExternalOutput")
    RG = [list(range(NCORES))]
    F32R = mybir.dt.float32r
    bar_in = nc.dram_tensor("bar_in", [1, B], F32R)
    bar_out = nc.dram_tensor("bar_out", [NCORES, B], F32R)

    # Long-lived constants/weights: static SBUF allocations.
    id128f = nc.alloc_sbuf_tensor("id128f", [128, 128], F32).ap()
    id128b = nc.alloc_sbuf_tensor("id128b", [128, 128], BF16).ap()
    b_sb = nc.alloc_sbuf_tensor("b_sb", [128, NG], F32).ap()
    wsb = [nc.alloc_sbuf_tensor(f"wsb{i}", [128, NS], BF16).ap()
           for i in range(IN // 128)]
    usb = [nc.alloc_sbuf_tensor(f"usb{j}", [128, NS], BF16).ap()
           for j in range(H // 128)]
    # xz^T, SBUF-resident: gate g's row-chunk at columns [g*TB, (g+1)*TB).
    xzb = nc.alloc_sbuf_tensor("xzb", [128, NG * t_steps * B], BF16).ap()
    xzb_v = xzb.rearrange("p (g c) -> p g c", g=NG)
    c_st = [nc.alloc_sbuf_tensor(f"c_st{i}", [128, B], F32).ap()
            for i in range(2)]
    # Receive buffers: core j's h^T slice lands at columns [j*B, (j+1)*B).
    # Last 8 cols are a local-write pad threading Tile deps (see critical).
    hT_recv = [
        nc.alloc_sbuf_tensor(f"hTr{i}", [128, NCORES * B + 8], BF16).ap()
        for i in range(2)
    ]
    htx = [nc.alloc_sbuf_tensor(f"htx{i}", [128, B], BF16).ap()
           for i in range(2)]
    hring = [nc.alloc_sbuf_tensor(f"hring{i}", [128, RSTEPS * B], F32).ap()
             for i in range(2)]

    with tile.TileContext(nc) as tc:
        with (
            tc.tile_pool(name="xin", bufs=3) as xin_pool,
            tc.tile_pool(name="xts", bufs=2) as xts_pool,
            tc.tile_pool(name="gts", bufs=2) as g_pool,
            tc.tile_pool(name="psA", bufs=2, space=bass.MemorySpace.PSUM) as psA_pool,
            tc.tile_pool(name="psT", bufs=2, space=bass.MemorySpace.PSUM) as psT_pool,
            tc.tile_pool(name="psG", bufs=1, space=bass.MemorySpace.PSUM) as psG_pool,
        ):
            nc.any.memset(c_st[0], 0.0)
            make_identity(nc, id128f)
            nc.vector.tensor_copy(id128b, id128f)
            nc.gpsimd.dma_start(b_sb, b_d.ap())
            for i in range(IN // 128):
                nc.gpsimd.dma_start(wsb[i], w_d.ap()[ts(i, 128), :])
            for j in range(H // 128):
                nc.gpsimd.dma_start(usb[j], u_d.ap()[ts(j, 128), :])

            exchange = ("noex" not in ablate) and not sim_local
            if exchange:
                # Prologue: clear cross-core sems, then barrier so no core's
                # step-0 credit can arrive before a peer's clear.
                arrv = nc.alloc_semaphore("arrv")
                lsem = nc.alloc_semaphore("rdl_sem")
                cls = [nc.gpsimd.sem_clear(s) for s in (arrv, lsem)]
                barz = nc.inline_tensor(np.zeros((1, B), np.float32), name="barz")
                bz = nc.gpsimd.dma_start(bar_in.ap().bitcast(F32), barz.ap())
                bar = nc.gpsimd.collective_compute(
                    "AllGather",
                    mybir.AluOpType.bypass,
                    replica_groups=RG,
                    ins=[bar_in.ap().opt()],
                    outs=[bar_out.ap().opt()],
                )
                for cl in cls:
                    add_dep_helper(bar.ins, cl.ins, reason="barrier after sem clear")
                add_dep_helper(bar.ins, bz.ins, reason="barrier after input init")
                pid = nc.gpsimd.partition_id()
                pofs = pid * B
                prev_gp = bar
            g_ex = 0

            def emit_phase_a_tile(m):
                # xzb[:, g, m*128:(m+1)*128] = (x_tile @ W)^T, one 128-row tile
                xin = xin_pool.tile([128, IN], BF16, tag="xin")
                nc.sync.dma_start(xin, x_d.ap()[ts(m, 128), :])
                zp = psA_pool.tile([128, NS], F32, tag="zpa")
                NI = IN // 128
                for ic in range(NI):
                    # Full-bank psum tile so two in-flight transposes never
                    # share a bank (PE-write + DVE-read same bank is fatal).
                    xTp = psT_pool.tile([128, 1024], BF16, tag="xTp")
                    nc.tensor.transpose(xTp[:, 0:128], xin[:, ts(ic, 128)], id128b)
                    xTs = xts_pool.tile([128, 128], BF16, tag=f"xts{ic}")
                    nc.vector.tensor_copy(xTs, xTp[:, 0:128])
                    for gc in range(NG):
                        # zp is ONE psum bank: a single start=True clears it;
                        # per-element has_written bits make later first-writes
                        # to other column ranges overwrite, repeats accumulate.
                        nc.tensor.matmul(
                            zp[:, ts(gc, 128)], wsb[ic][:, ts(gc, 128)], xTs,
                            start=(ic == 0 and gc == 0),
                            stop=(ic == NI - 1 and gc == NG - 1),
                        )
                nc.vector.tensor_copy(
                    xzb_v[:, :, ts(m, 128)],
                    zp.rearrange("p (g c) -> p g c", g=NG),
                )

            def emit_injects(t):
                # xz^T inject: starts each gate's PSUM accumulation; emitted a
                # step early so it runs inside the exchange-wait gap.
                zg = []
                for g in range(NG):
                    # Full-bank tile: each gate must own its bank (ACT reads
                    # gate i while PE writes gate o -> same bank is fatal).
                    zf = psG_pool.tile([128, 512], F32, tag=f"zg{g}")
                    z = zf[:, 0:B]
                    nc.tensor.matmul(
                        z, id128b, xzb_v[:, g, ds(t * B, B)],
                        start=True, stop=(t == 0 or "nomm" in ablate),
                        skip_group_check=True,
                    )
                    zg.append(z)
                return zg

            ntiles = t_steps * B // 128

            for _rep in range(reps):
                for m in range(min(2, ntiles)):
                    emit_phase_a_tile(m)
                if "aonly" in ablate:
                    for m in range(2, ntiles):
                        emit_phase_a_tile(m)
                    continue

                zg = emit_injects(0)
                hT_prev = None
                for t in range(t_steps):
                    last = t == t_steps - 1
                    buf = t % 2
                    if exchange and not last:
                        # Descgen early: encodes only addresses; htx data is
                        # read at trigger time, gated on the cast below.
                        prep = nc.gpsimd.remote_dma_broadcast(
                            hT_recv[buf][:, ds(pofs, B)],
                            htx[buf][:, :],
                            remote_sem=arrv,
                            local_sem=lsem,
                            rdests=[(0, m) for m in range(NCORES)],
                        )
                        add_dep_helper(
                            prep.ins, prev_gp.ins,
                            reason="SWDGE FIFO: prep after prev trig",
                        )
                        prev_gp = prep

                    if t > 0 and "nomm" not in ablate:
                        # Gate order i, g, f, o: the c-chain (needs i,g,f)
                        # hides under o's matmuls; tail is just sig_o+tanh_c+h.
                        for g in (0, 2, 1, 3):
                            for j in range(H // 128):
                                nc.tensor.matmul(
                                    zg[g], usb[j][:, ts(g, 128)],
                                    hT_prev[:, ds(j * B, B)],
                                    start=False, stop=(j == H // 128 - 1),
                                    skip_group_check=True,
                                )

                    # Tail: activations + elementwise in [128, 32] layout.
                    # Gate order in PSUM: 0=i, 1=f, 2=g, 3=o.
                    sig_i = g_pool.tile([128, B], F32, tag="sig_i")
                    nc.scalar.activation(sig_i, zg[0], AF.Sigmoid,
                                         bias=b_sb[:, 0:1])
                    tan_g = g_pool.tile([128, B], F32, tag="tan_g")
                    nc.scalar.activation(tan_g, zg[2], AF.Tanh,
                                         bias=b_sb[:, 2:3])
                    ig = g_pool.tile([128, B], F32, tag="ig")
                    nc.vector.tensor_mul(ig, sig_i, tan_g)
                    sig_f = g_pool.tile([128, B], F32, tag="sig_f")
                    nc.scalar.activation(sig_f, zg[1], AF.Sigmoid,
                                         bias=b_sb[:, 1:2])
                    fc = g_pool.tile([128, B], F32, tag="fc")
                    nc.vector.tensor_mul(fc, sig_f, c_st[t % 2])
                    c_new = c_st[(t + 1) % 2]
                    nc.vector.tensor_add(c_new, ig, fc)
                    sig_o = g_pool.tile([128, B], F32, tag="sig_o")
                    nc.scalar.activation(sig_o, zg[3], AF.Sigmoid,
                                         bias=b_sb[:, 3:4])
                    tc_t = g_pool.tile([128, B], F32, tag="tc")
                    nc.scalar.activation(tc_t, c_new, AF.Tanh)
                    h_t = hring[(t // RSTEPS) % 2][:, ds((t % RSTEPS) * B, B)]

                    cp = None
                    if not last:
                        # Serial-path h: one DVE mul straight into the bf16
                        # staging tile; the fp32 ring copy is off-path.
                        cp = nc.vector.tensor_mul(htx[buf], sig_o, tc_t)
                        g_ex += 1
                    nc.vector.tensor_mul(h_t, sig_o, tc_t)

                    if t % RSTEPS == RSTEPS - 1 or last:
                        n_fl = t % RSTEPS + 1
                        t0f = t - n_fl + 1
                        nc.sync.dma_start(
                            hs_d.ap()[:, ds(t0f * B, n_fl * B)],
                            hring[(t // RSTEPS) % 2][:, 0:n_fl * B],
                        )

                    # PE gap-filler work for the upcoming exchange wait:
                    m_a = t // 2 + 2
                    if t % 2 == 0 and m_a < ntiles:
                        emit_phase_a_tile(m_a)
                    if not last:
                        zg = emit_injects(t + 1)

                    if not last:
                        if sim_local:
                            nc.gpsimd.dma_start(
                                hT_recv[buf][:, ds(0, B)], htx[buf])
                            hT_prev = hT_recv[buf]
                        elif exchange:
                            trig = nc.gpsimd.trigger_dma(count=1)
                            add_dep_helper(
                                trig.ins, cp.ins,
                                reason="fire broadcast after h^T staged",
                            )
                            add_dep_helper(
                                trig.ins, prev_gp.ins,
                                reason="trigger order matches prep order",
                            )
                            prev_gp = trig
                            with tc.tile_critical(name=f"hx{t}"):
                                nc.vector.tensor_copy(
                                    hT_recv[buf][0:1, PADC:PADC + 1],
                                    htx[buf][0:1, 0:1],
                                )
                                wt = nc.sync.wait_ge(arrv, 16 * g_ex)
                                add_dep_helper(
                                    wt.ins, trig.ins,
                                    reason="own trigger before blocking wait",
                                )
                                if g_ex == 1:
                                    add_dep_helper(
                                        wt.ins, bar.ins,
                                        reason="first wait after barrier",
                                    )
                            hT_prev = hT_recv[buf]
                        else:  # noex ablation: stale data, timing only
                            hT_prev = hT_recv[buf]

    nc.compile()
    return nc


def _make_in_maps(x, W, U, b, t_steps: int = T):
    bf = ml_dtypes.bfloat16
    x = np.asarray(x, np.float32)[:, :t_steps, :]
    xt = np.ascontiguousarray(np.swapaxes(x, 0, 1)).reshape(t_steps * B, IN)
    W = np.asarray(W, np.float32)
    U = np.asarray(U, np.float32)
    b = np.asarray(b, np.float32)
    in_maps = []
    for k in range(NCORES):
        # per-core gate column order: [i | f | g | o], H-slice k
        cols = np.concatenate(
            [np.arange(k * HS, (k + 1) * HS) + g * H for g in range(NG)]
        )
        in_maps.append(
            {
                "x": xt.astype(bf),
                "w": np.ascontiguousarray(W[:, cols]).astype(bf),
                "u": np.ascontiguousarray(U[:, cols]).astype(bf),
                "b": np.ascontiguousarray(b[cols].reshape(NG, HS).T),
            }
        )
    return in_maps


def _pjrt_bundle(nc, n_reps: int = 1):
    """Reusable sharded PJRT executable (see v1 docstring)."""
    import jax
    from jax.experimental.shard_map import shard_map
    from jax.sharding import Mesh, PartitionSpec
    from concourse import bass2jax

    bass2jax.install_neuronx_cc_hook()
    partition_name = nc.partition_id_tensor.name if nc.partition_id_tensor else None
    in_names, out_names, out_avals, zero_outs = [], [], [], []
    for alloc in nc.m.functions[0].allocations:
        if not isinstance(alloc, mybir.MemoryLocationSet):
            continue
        name = alloc.memorylocations[0].name
        if alloc.kind == "ExternalInput":
            if name != partition_name:
                in_names.append(name)
        elif alloc.kind == "ExternalOutput":
            shape = tuple(alloc.tensor_shape)
            dtype = mybir.dt.np(alloc.dtype)
            out_names.append(name)
            out_avals.append(jax.core.ShapedArray(shape, dtype))
            zero_outs.append(np.zeros(shape, dtype))
    n_params = len(in_names)
    n_outs = len(out_avals)
    all_in_names = list(in_names) + list(out_names)
    if partition_name is not None:
        all_in_names.append(partition_name)

    def _body(*args):
        ins = list(args[:n_params])
        zs = list(args[n_params:])
        for _ in range(n_reps):
            operands = ins + zs
            if partition_name is not None:
                operands.append(bass2jax.partition_id_tensor())
            outs = bass2jax._bass_exec_p.bind(
                *operands,
                out_avals=tuple(out_avals),
                in_names=tuple(all_in_names),
                out_names=tuple(out_names),
                lowering_input_output_aliases=(),
                sim_require_finite=True,
                sim_require_nnan=True,
                nc=nc,
            )
            zs = list(outs)
        return tuple(outs)

    devices = jax.devices()[:NCORES]
    mesh = Mesh(np.asarray(devices), ("core",))
    in_specs = (PartitionSpec("core"),) * (n_params + n_outs)
    out_specs = (PartitionSpec("core"),) * n_outs
    sharded = jax.jit(
        shard_map(
            _body, mesh=mesh, in_specs=in_specs, out_specs=out_specs, check_rep=False
        ),
        donate_argnums=tuple(range(n_params, n_params + n_outs)),
        keep_unused=True,
    )
    return dict(
        fn=sharded,
        mesh=mesh,
        in_names=in_names,
        out_names=out_names,
        out_avals=out_avals,
        zero_outs=zero_outs,
        n_params=n_params,
    )


def assemble(hs_list, t_steps: int = T):
    """hs_list[k]: [128, t_steps*B] f32 (h^T-major) -> full [B, T, H]."""
    out = np.empty((B, t_steps, H), np.float32)
    for k in range(NCORES):
        hk = np.asarray(hs_list[k]).reshape(HS, t_steps, B)
        out[:, :, k * HS:(k + 1) * HS] = hk.transpose(2, 1, 0)
    return out


def _run(inputs, t_steps: int = T, trace: bool = False):
    nc = _build(t_steps)
    in_maps = _make_in_maps(inputs["x"], inputs["W"], inputs["U"], inputs["b"], t_steps)
    res = bass_utils.run_bass_kernel_spmd(
        nc, in_maps, core_ids=list(range(NCORES)), trace=trace
    )
    out = assemble([res.results[k]["hs"] for k in range(NCORES)], t_steps)
    return out, res


def kernel(**inputs) -> np.ndarray:
    out, _ = _run(inputs)
    return out


# revision 3
# speedup vs baseline: 6.4573x; 6.4573x over previous
"""TRN2 Bass kernel v2 for BasicLSTM (B=32, T=512, IN=512, H=1024).

Strategy vs v1: the wall time of the recurrence is 512 serial steps x
per-step latency L; chains/batch-splits cannot reduce it, only L can.
So v2 minimizes L with a TRANSPOSED single-chain layout:

  - Core k owns gate columns [i_k | f_k | g_k | o_k] (H-slice k*128..),
    and computes z^T = U_k^T h + xz^T directly: out tiles are
    [128 gate-rows, 32 batch] -- full 128-partition utilization for the
    activations/elementwise, and h^T [128, 32] is produced in exactly
    the layout the broadcast needs (no PE transpose on the serial path).
  - U is the STATIONARY operand (bf16 -> fast weight load), h^T bf16 is
    the moving operand: 4 gate-chunks x 8 K-chunks = 32 small matmuls
    per step, LDW-bound at ~55-70ns each.
  - xz^T is precomputed by phase A into SBUF-resident bf16 (128KB/part):
    no per-step DRAM traffic; injected into each gate's PSUM bank with
    an identity matmul (start=True) emitted a step EARLY so it runs in
    the exchange-wait gap.  Bias is applied for free via the ACT bias
    port (per-partition in the transposed layout).
  - Exchange: remote_dma_broadcast of h^T bf16 [128, 32] (descgen-early
    prep + trigger, cumulative credit counting, wait in tile_critical)
    -- same machinery as v1 but once per step instead of twice.
  - Output: h kept fp32 in a 32-step SBUF ring, bulk-DMA'd to DRAM in
    h^T-major layout [128, T, B]; the host transposes when assembling.
  - Numerics: x/W/xz/U/h in bf16, PSUM/activations/c/h-out in fp32.
    Host-emulated rel err ~5e-3 (gate is 2e-2).
"""

import numpy as np
import ml_dtypes

import concourse.bass as bass
import concourse.mybir as mybir
import concourse.tile as tile
from concourse import bacc, bass_utils
from concourse.bass import ts, ds
from concourse.masks import make_identity
from bass_rust import add_dep_helper

B = 32                # batch (single chain, all rows)
T = 512
IN = 512
H = 1024
NCORES = 8
NS = 4 * H // NCORES  # 512 gate cols per core
HS = H // NCORES      # 128 h cols per core
NG = 4                # gates per core (i, f, g, o), each an HS-chunk
TB = T * B
F32 = mybir.dt.float32
BF16 = mybir.dt.bfloat16
AF = mybir.ActivationFunctionType
RSTEPS = 32           # h output ring: steps buffered before one bulk DMA
PADC = NCORES * B     # pad column in hT_recv (Tile dep threading)
WARM = 10             # dummy matmuls per exchange gap: keep the PE-HAM clock
                      # gate at 8/8 (idle > ~3.4us re-throttles PE to 1.2GHz;
                      # PE-mode transposes do NOT count as HAM activity)


def _build(t_steps: int = T, ablate: frozenset = frozenset(), reps: int = 1,
           sim_local: bool = False):
    """ablate (perf experiments only, breaks numerics):
    'aonly' - phase A only;  'noex' - skip broadcasts + waits (stale hT);
    'nomm'  - skip the recurrent matmuls.
    sim_local: replace the cross-core exchange with a local SBUF DMA so a
    single-core TimelineSim run has an equivalent serial dependency."""
    assert t_steps % 4 == 0
    nc = bacc.Bacc("TRN2", debug=False, num_devices=NCORES)

    x_d = nc.dram_tensor("x", [t_steps * B, IN], BF16, kind="ExternalInput")
    w_d = nc.dram_tensor("w", [IN, NS], BF16, kind="ExternalInput")
    u_d = nc.dram_tensor("u", [H, NS], BF16, kind="ExternalInput")
    b_d = nc.dram_tensor("b", [128, NG], F32, kind="ExternalInput")
    hs_d = nc.dram_tensor("hs", [128, t_steps * B], F32, kind="ExternalOutput")
    RG = [list(range(NCORES))]
    F32R = mybir.dt.float32r
    bar_in = nc.dram_tensor("bar_in", [1, B], F32R)
    bar_out = nc.dram_tensor("bar_out", [NCORES, B], F32R)

    # Long-lived constants/weights: static SBUF allocations.
    id128f = nc.alloc_sbuf_tensor("id128f", [128, 128], F32).ap()
    id128b = nc.alloc_sbuf_tensor("id128b", [128, 128], BF16).ap()
    b_sb = nc.alloc_sbuf_tensor("b_sb", [128, NG], F32).ap()
    wsb = [nc.alloc_sbuf_tensor(f"wsb{i}", [128, NS], BF16).ap()
           for i in range(IN // 128)]
    usb = [nc.alloc_sbuf_tensor(f"usb{j}", [128, NS], BF16).ap()
           for j in range(H // 128)]
    # xz^T, SBUF-resident: gate g's row-chunk at columns [g*TB, (g+1)*TB).
    xzb = nc.alloc_sbuf_tensor("xzb", [128, NG * t_steps * B], BF16).ap()
    xzb_v = xzb.rearrange("p (g c) -> p g c", g=NG)
    c_st = [nc.alloc_sbuf_tensor(f"c_st{i}", [128, B], F32).ap()
            for i in range(2)]
    # Receive buffers: core j's h^T slice lands at columns [j*B, (j+1)*B).
    # Last 8 cols are a local-write pad threading Tile deps (see critical).
    hT_recv = [
        nc.alloc_sbuf_tensor(f"hTr{i}", [128, NCORES * B + 8], BF16).ap()
        for i in range(2)
    ]
    htx = [nc.alloc_sbuf_tensor(f"htx{i}", [128, B], BF16).ap()
           for i in range(2)]
    hring = [nc.alloc_sbuf_tensor(f"hring{i}", [128, RSTEPS * B], F32).ap()
             for i in range(2)]

    with tile.TileContext(nc) as tc:
        with (
            tc.tile_pool(name="xin", bufs=3) as xin_pool,
            tc.tile_pool(name="xts", bufs=2) as xts_pool,
            tc.tile_pool(name="gts", bufs=2) as g_pool,
            tc.tile_pool(name="psA", bufs=2, space=bass.MemorySpace.PSUM) as psA_pool,
            tc.tile_pool(name="psT", bufs=2, space=bass.MemorySpace.PSUM) as psT_pool,
            tc.tile_pool(name="psG", bufs=1, space=bass.MemorySpace.PSUM) as psG_pool,
        ):
            nc.any.memset(c_st[0], 0.0)
            make_identity(nc, id128f)
            nc.vector.tensor_copy(id128b, id128f)
            nc.gpsimd.dma_start(b_sb, b_d.ap())
            for i in range(IN // 128):
                nc.gpsimd.dma_start(wsb[i], w_d.ap()[ts(i, 128), :])
            for j in range(H // 128):
                nc.gpsimd.dma_start(usb[j], u_d.ap()[ts(j, 128), :])

            exchange = ("noex" not in ablate) and not sim_local
            if exchange:
                # Prologue: clear cross-core sems, then barrier so no core's
                # step-0 credit can arrive before a peer's clear.
                arrv = nc.alloc_semaphore("arrv")
                lsem = nc.alloc_semaphore("rdl_sem")
                cls = [nc.gpsimd.sem_clear(s) for s in (arrv, lsem)]
                barz = nc.inline_tensor(np.zeros((1, B), np.float32), name="barz")
                bz = nc.gpsimd.dma_start(bar_in.ap().bitcast(F32), barz.ap())
                bar = nc.gpsimd.collective_compute(
                    "AllGather",
                    mybir.AluOpType.bypass,
                    replica_groups=RG,
                    ins=[bar_in.ap().opt()],
                    outs=[bar_out.ap().opt()],
                )
                for cl in cls:
                    add_dep_helper(bar.ins, cl.ins, reason="barrier after sem clear")
                add_dep_helper(bar.ins, bz.ins, reason="barrier after input init")
                pid = nc.gpsimd.partition_id()
                pofs = pid * B
                prev_gp = bar
            g_ex = 0

            def emit_phase_a_tile(m):
                # xzb[:, g, m*128:(m+1)*128] = (x_tile @ W)^T, one 128-row tile
                xin = xin_pool.tile([128, IN], BF16, tag="xin")
                nc.sync.dma_start(xin, x_d.ap()[ts(m, 128), :])
                zp = psA_pool.tile([128, NS], F32, tag="zpa")
                NI = IN // 128
                for ic in range(NI):
                    # Full-bank psum tile so two in-flight transposes never
                    # share a bank (PE-write + DVE-read same bank is fatal).
                    xTp = psT_pool.tile([128, 1024], BF16, tag="xTp")
                    nc.tensor.transpose(xTp[:, 0:128], xin[:, ts(ic, 128)], id128b)
                    xTs = xts_pool.tile([128, 128], BF16, tag=f"xts{ic}")
                    nc.vector.tensor_copy(xTs, xTp[:, 0:128])
                    for gc in range(NG):
                        # zp is ONE psum bank: a single start=True clears it;
                        # per-element has_written bits make later first-writes
                        # to other column ranges overwrite, repeats accumulate.
                        nc.tensor.matmul(
                            zp[:, ts(gc, 128)], wsb[ic][:, ts(gc, 128)], xTs,
                            start=(ic == 0 and gc == 0),
                            stop=(ic == NI - 1 and gc == NG - 1),
                        )
                nc.vector.tensor_copy(
                    xzb_v[:, :, ts(m, 128)],
                    zp.rearrange("p (g c) -> p g c", g=NG),
                )

            def emit_injects(t):
                # xz^T inject: starts each gate's PSUM accumulation; emitted a
                # step early so it runs inside the exchange-wait gap.
                zg = []
                for g in range(NG):
                    # Full-bank tile: each gate must own its bank (ACT reads
                    # gate i while PE writes gate o -> same bank is fatal).
                    zf = psG_pool.tile([128, 512], F32, tag=f"zg{g}")
                    z = zf[:, 0:B]
                    nc.tensor.matmul(
                        z, id128b, xzb_v[:, g, ds(t * B, B)],
                        start=True, stop=(t == 0 or "nomm" in ablate),
                        skip_group_check=True,
                    )
                    zg.append(z)
                return zg

            ntiles = t_steps * B // 128

            for _rep in range(reps):
                for m in range(min(2, ntiles)):
                    emit_phase_a_tile(m)
                if "aonly" in ablate:
                    for m in range(2, ntiles):
                        emit_phase_a_tile(m)
                    continue

                zg = emit_injects(0)
                hT_prev = None
                for t in range(t_steps):
                    last = t == t_steps - 1
                    buf = t % 2
                    if exchange and not last:
                        # Descgen early: encodes only addresses; htx data is
                        # read at trigger time, gated on the cast below.
                        prep = nc.gpsimd.remote_dma_broadcast(
                            hT_recv[buf][:, ds(pofs, B)],
                            htx[buf][:, :],
                            remote_sem=arrv,
                            local_sem=lsem,
                            rdests=[(0, m) for m in range(NCORES)],
                        )
                        add_dep_helper(
                            prep.ins, prev_gp.ins,
                            reason="SWDGE FIFO: prep after prev trig",
                        )
                        prev_gp = prep

                    if t > 0 and "nomm" not in ablate:
                        # Gate order i, g, f, o: the c-chain (needs i,g,f)
                        # hides under o's matmuls; tail is just sig_o+tanh_c+h.
                        for g in (0, 2, 1, 3):
                            for j in range(H // 128):
                                nc.tensor.matmul(
                                    zg[g], usb[j][:, ts(g, 128)],
                                    hT_prev[:, ds(j * B, B)],
                                    start=False, stop=(j == H // 128 - 1),
                                    skip_group_check=True,
                                )

                    # Tail: activations + elementwise in [128, 32] layout.
                    # Gate order in PSUM: 0=i, 1=f, 2=g, 3=o.
                    sig_i = g_pool.tile([128, B], F32, tag="sig_i")
                    nc.scalar.activation(sig_i, zg[0], AF.Sigmoid,
                                         bias=b_sb[:, 0:1])
                    tan_g = g_pool.tile([128, B], F32, tag="tan_g")
                    nc.scalar.activation(tan_g, zg[2], AF.Tanh,
                                         bias=b_sb[:, 2:3])
                    ig = g_pool.tile([128, B], F32, tag="ig")
                    nc.vector.tensor_mul(ig, sig_i, tan_g)
                    sig_f = g_pool.tile([128, B], F32, tag="sig_f")
                    nc.scalar.activation(sig_f, zg[1], AF.Sigmoid,
                                         bias=b_sb[:, 1:2])
                    fc = g_pool.tile([128, B], F32, tag="fc")
                    nc.vector.tensor_mul(fc, sig_f, c_st[t % 2])
                    c_new = c_st[(t + 1) % 2]
                    nc.vector.tensor_add(c_new, ig, fc)
                    sig_o = g_pool.tile([128, B], F32, tag="sig_o")
                    nc.scalar.activation(sig_o, zg[3], AF.Sigmoid,
                                         bias=b_sb[:, 3:4])
                    tc_t = g_pool.tile([128, B], F32, tag="tc")
                    nc.scalar.activation(tc_t, c_new, AF.Tanh)
                    h_t = hring[(t // RSTEPS) % 2][:, ds((t % RSTEPS) * B, B)]

                    cp = None
                    if not last:
                        # Serial-path h: one DVE mul straight into the bf16
                        # staging tile; the fp32 ring copy is off-path.
                        cp = nc.vector.tensor_mul(htx[buf], sig_o, tc_t)
                        g_ex += 1
                    nc.vector.tensor_mul(h_t, sig_o, tc_t)

                    if t % RSTEPS == RSTEPS - 1 or last:
                        n_fl = t % RSTEPS + 1
                        t0f = t - n_fl + 1
                        nc.sync.dma_start(
                            hs_d.ap()[:, ds(t0f * B, n_fl * B)],
                            hring[(t // RSTEPS) % 2][:, 0:n_fl * B],
                        )

                    # PE gap-filler work for the upcoming exchange wait:
                    m_a = t // 3 + 2
                    if t % 3 == 0 and m_a < ntiles:
                        emit_phase_a_tile(m_a)
                    elif not last:
                        wp = psA_pool.tile([128, NS], F32, tag="zpa")
                        for w in range(WARM):
                            nc.tensor.matmul(
                                wp[:, 0:128], id128b, id128b,
                                start=(w == 0), stop=(w == WARM - 1),
                            )
                    if not last:
                        zg = emit_injects(t + 1)

                    if not last:
                        if sim_local:
                            nc.gpsimd.dma_start(
                                hT_recv[buf][:, ds(0, B)], htx[buf])
                            hT_prev = hT_recv[buf]
                        elif exchange:
                            trig = nc.gpsimd.trigger_dma(count=1)
                            add_dep_helper(
                                trig.ins, cp.ins,
                                reason="fire broadcast after h^T staged",
                            )
                            add_dep_helper(
                                trig.ins, prev_gp.ins,
                                reason="trigger order matches prep order",
                            )
                            prev_gp = trig
                            with tc.tile_critical(name=f"hx{t}"):
                                nc.vector.tensor_copy(
                                    hT_recv[buf][0:1, PADC:PADC + 1],
                                    htx[buf][0:1, 0:1],
                                )
                                wt = nc.sync.wait_ge(arrv, 16 * g_ex)
                                add_dep_helper(
                                    wt.ins, trig.ins,
                                    reason="own trigger before blocking wait",
                                )
                                if g_ex == 1:
                                    add_dep_helper(
                                        wt.ins, bar.ins,
                                        reason="first wait after barrier",
                                    )
                            hT_prev = hT_recv[buf]
                        else:  # noex ablation: stale data, timing only
                            hT_prev = hT_recv[buf]

    nc.compile()
    return nc


def _make_in_maps(x, W, U, b, t_steps: int = T):
    bf = ml_dtypes.bfloat16
    x = np.asarray(x, np.float32)[:, :t_steps, :]
    xt = np.ascontiguousarray(np.swapaxes(x, 0, 1)).reshape(t_steps * B, IN)
    W = np.asarray(W, np.float32)
    U = np.asarray(U, np.float32)
    b = np.asarray(b, np.float32)
    in_maps = []
    for k in range(NCORES):
        # per-core gate column order: [i | f | g | o], H-slice k
        cols = np.concatenate(
            [np.arange(k * HS, (k + 1) * HS) + g * H for g in range(NG)]
        )
        in_maps.append(
            {
                "x": xt.astype(bf),
                "w": np.ascontiguousarray(W[:, cols]).astype(bf),
                "u": np.ascontiguousarray(U[:, cols]).astype(bf),
                "b": np.ascontiguousarray(b[cols].reshape(NG, HS).T),
            }
        )
    return in_maps


def _pjrt_bundle(nc, n_reps: int = 1):
    """Reusable sharded PJRT executable (see v1 docstring)."""
    import jax
    from jax.experimental.shard_map import shard_map
    from jax.sharding import Mesh, PartitionSpec
    from concourse import bass2jax

    bass2jax.install_neuronx_cc_hook()
    partition_name = nc.partition_id_tensor.name if nc.partition_id_tensor else None
    in_names, out_names, out_avals, zero_outs = [], [], [], []
    for alloc in nc.m.functions[0].allocations:
        if not isinstance(alloc, mybir.MemoryLocationSet):
            continue
        name = alloc.memorylocations[0].name
        if alloc.kind == "ExternalInput":
            if name != partition_name:
                in_names.append(name)
        elif alloc.kind == "ExternalOutput":
            shape = tuple(alloc.tensor_shape)
            dtype = mybir.dt.np(alloc.dtype)
            out_names.append(name)
            out_avals.append(jax.core.ShapedArray(shape, dtype))
            zero_outs.append(np.zeros(shape, dtype))
    n_params = len(in_names)
    n_outs = len(out_avals)
    all_in_names = list(in_names) + list(out_names)
    if partition_name is not None:
        all_in_names.append(partition_name)

    def _body(*args):
        ins = list(args[:n_params])
        zs = list(args[n_params:])
        for _ in range(n_reps):
            operands = ins + zs
            if partition_name is not None:
                operands.append(bass2jax.partition_id_tensor())
            outs = bass2jax._bass_exec_p.bind(
                *operands,
                out_avals=tuple(out_avals),
                in_names=tuple(all_in_names),
                out_names=tuple(out_names),
                lowering_input_output_aliases=(),
                sim_require_finite=True,
                sim_require_nnan=True,
                nc=nc,
            )
            zs = list(outs)
        return tuple(outs)

    devices = jax.devices()[:NCORES]
    mesh = Mesh(np.asarray(devices), ("core",))
    in_specs = (PartitionSpec("core"),) * (n_params + n_outs)
    out_specs = (PartitionSpec("core"),) * n_outs
    sharded = jax.jit(
        shard_map(
            _body, mesh=mesh, in_specs=in_specs, out_specs=out_specs, check_rep=False
        ),
        donate_argnums=tuple(range(n_params, n_params + n_outs)),
        keep_unused=True,
    )
    return dict(
        fn=sharded,
        mesh=mesh,
        in_names=in_names,
        out_names=out_names,
        out_avals=out_avals,
        zero_outs=zero_outs,
        n_params=n_params,
    )


def assemble(hs_list, t_steps: int = T):
    """hs_list[k]: [128, t_steps*B] f32 (h^T-major) -> full [B, T, H]."""
    out = np.empty((B, t_steps, H), np.float32)
    for k in range(NCORES):
        hk = np.asarray(hs_list[k]).reshape(HS, t_steps, B)
        out[:, :, k * HS:(k + 1) * HS] = hk.transpose(2, 1, 0)
    return out


def _run(inputs, t_steps: int = T, trace: bool = False):
    nc = _build(t_steps)
    in_maps = _make_in_maps(inputs["x"], inputs["W"], inputs["U"], inputs["b"], t_steps)
    res = bass_utils.run_bass_kernel_spmd(
        nc, in_maps, core_ids=list(range(NCORES)), trace=trace
    )
    out = assemble([res.results[k]["hs"] for k in range(NCORES)], t_steps)
    return out, res


def kernel(**inputs) -> np.ndarray:
    out, _ = _run(inputs)
    return out
